# revision 78
# baseline (speedup 1.0000x reference)
"""Trainium2 Bass kernel for causal MHA (b=2, n=4096, d_model=768, 12 heads).

Sharding: 8 cores = 2 batches x 4 head-groups (3 heads each).
Each core:
  - receives its batch's Q/K/V pre-transposed ([768, n], d_model on rows)
    plus its head-group's weight slices (also pre-transposed on host).
  - projects qT/kT ([64, n] per head, head dim on partitions) and
    v ([n, 64] per head, tokens on partitions) on-chip.
  - computes scoresT[k, q] = kT^T @ qT for PAIRS of 128-key blocks into a
    single [128, 1024] two-bank PSUM tile, exponentiates the pair with ONE
    activation instruction (halves the per-instruction ACT overhead),
    masks causal-boundary blocks with a precomputed 0/1 mask, and
    accumulates outT_aug[65, q] += [v | ones]^T @ P in PSUM.  Row 64 is
    the softmax denominator.
  - normalizes per query-chunk: 1/denom via the fast custom-DVE
    reciprocal, replicated across 64 partitions with a K=1 PE matmul
    (ones[1,64]^T @ r1[1,512] -> PSUM), then one tensor_mul into outT.
    No DRAM round-trips.
  - applies the output projection with its w_o row-slice; host sums the
    4 partial outputs per batch (row-parallel linear unshard).

The phases are software-pipelined: projection chunk t+1, v-projection
blocks, and the output projection for chunk j-1 are issued between
attention chunks so the PE fills the slack of the ACT-bound attention
inner loop and the HAM clock gate stays warm (2.4 GHz).

Weight-column host layout packs the six 64-wide q/k heads into three full
128-row M-blocks ([q0;q1], [q2;k2], [k0;k1]); k2/q2 are then DMA-copied
into a fourth block so every head's scores matmul sees its qT and kT at
the same partition base (a matmul constraint), with h2's operands at BOTH
bases so it can pair with whichever PE row-group is free.
"""

import sys

for _p in ("/opt/trn_rl_repo",):
    if _p not in sys.path:
        sys.path.insert(0, _p)

import numpy as np
import ml_dtypes

import concourse.bass as bass  # noqa: F401  (registers engine classes)
import concourse.tile as tile
from concourse import bacc, mybir
import concourse.bass_utils as bass_utils

P = 128
D_MODEL = 768
KO = D_MODEL // P  # 6 contraction chunks of 128
N_HEADS = 12
D_K = 64
N_CORES = 8
H_LOCAL = 3  # heads per core
D_LOCAL = H_LOCAL * D_K  # 192
B = 2
N_TOKENS = 4096
NQ = 512  # query-chunk size (one PSUM bank of fp32)
NT = 512  # token chunk for q/k projection

F32 = mybir.dt.float32
BF16 = mybir.dt.bfloat16
F32R = mybir.dt.float32r
F8 = mybir.dt.float8e4  # TRN e4m3, max normal 240

# exp(s + ln(1/8)): keeps exp outputs < 240/8 so they fit fp8 e4m3; the
# constant cancels exactly in the softmax division.
EXP_BIAS_FP8 = float(np.log(1.0 / 8.0))

DEBUG = False


def _mm(ap, flavor):
    """View an fp32 AP as the matmul input dtype."""
    if flavor == "f32r":
        return ap.bitcast(F32R)
    return ap


def build_nc(n=N_TOKENS, mm="bf16", dt_x=BF16, dt_pt=BF16, dt_acc=BF16):
    assert n % NQ == 0 and n % NT == 0 and n % P == 0
    # fp8 pt enables DoubleRow PV matmuls (key-block pair fused, K=256)
    dr = dt_pt == F8
    exp_bias = EXP_BIAS_FP8 if dr else 0.0
    nc = bacc.Bacc("TRN2", target_bir_lowering=False, debug=False,
                   num_devices=N_CORES)

    qt_d = nc.dram_tensor("qt", [D_MODEL, n], dt_x, kind="ExternalInput")
    kt_d = nc.dram_tensor("kt", [D_MODEL, n], dt_x, kind="ExternalInput")
    vt_d = nc.dram_tensor("vt", [D_MODEL, n], dt_x, kind="ExternalInput")
    wqk_d = nc.dram_tensor("wqk", [D_MODEL, 2 * D_LOCAL], dt_x,
                           kind="ExternalInput")
    wv_d = nc.dram_tensor("wv", [D_MODEL, D_LOCAL], dt_x, kind="ExternalInput")
    wo_d = nc.dram_tensor("wo", [D_LOCAL, D_MODEL], dt_x, kind="ExternalInput")
    cm_d = nc.dram_tensor("cmask", [P, P], dt_pt, kind="ExternalInput")
    # transposed output [d_model, n]: the host transposes after gather —
    # this lets the output projection keep w_o stationary (12 weight loads
    # total) and stream outT as the moving operand.
    y_d = nc.dram_tensor("y", [D_MODEL, n], F32, kind="ExternalOutput")
    if DEBUG:
        dbg_d = nc.dram_tensor("dbg", [3 * (n // NQ) * H_LOCAL, NQ], F32,
                               kind="ExternalOutput")

    qt_r = qt_d.ap().rearrange("(ko ki) t -> ki ko t", ki=P)
    kt_r = kt_d.ap().rearrange("(ko ki) t -> ki ko t", ki=P)
    vt_r = vt_d.ap().rearrange("(ko ki) t -> ki ko t", ki=P)
    wqk_r = wqk_d.ap().rearrange("(ko ki) m -> ki ko m", ki=P)
    wv_r = wv_d.ap().rearrange("(ko ki) m -> ki ko m", ki=P)

    TCH = n // NT       # q/k projection token chunks
    TB = n // P         # 128-token blocks
    QCH = n // NQ       # query chunks
    KB_PER_Q = NQ // P  # key blocks per query chunk (4)

    # Host weight-column order: [q0 q1 | q2 k2 | k0 k1] -> 3 full M-blocks.
    # qkT_sb blk3 holds DMA-shifted copies: [0:64]=k2, [64:128]=q2.
    q_loc = {0: (0, 0), 1: (64, 0), 2: (0, 1)}
    k_loc = {0: (0, 2), 1: (64, 2), 2: (0, 3)}

    # one raw PSUM bank for HAM keep-alive dummy matmuls (outside the
    # tile pools; only the PE ever touches it, in queue order)
    ka_t = nc.alloc_psum_tensor("keepalive", [64, 96], F32)

    with tile.TileContext(nc) as tc:
        with tc.tile_pool(name="const", bufs=1) as cpool, \
             tc.tile_pool(name="persist", bufs=1) as ppool, \
             tc.tile_pool(name="xqk", bufs=4) as xpool, \
             tc.tile_pool(name="xv", bufs=6) as xvpool, \
             tc.tile_pool(name="pt", bufs=9) as ptpool, \
             tc.tile_pool(name="ysb", bufs=2) as ypool, \
             tc.tile_pool(name="rcp", bufs=4) as rpool, \
             tc.tile_pool(name="ot", bufs=2) as otpool, \
             tc.tile_pool(name="pp_sc", bufs=2, space="PSUM") as pp_sc, \
             tc.tile_pool(name="pp_out", bufs=1, space="PSUM") as pp_out:

            # ---- constants ----
            # DMA issue order = sync-queue order: the first projection
            # needs wqk and the first q/k chunk, so those go first; wv/wo/
            # cm follow behind them.
            wqk_sb = cpool.tile([P, KO, 2 * D_LOCAL], dt_x)
            nc.sync.dma_start(wqk_sb[:], wqk_r)
            wv_sb = cpool.tile([P, KO, D_LOCAL], dt_x)
            wo_sb = cpool.tile([P, 2, D_MODEL], dt_x)
            cm_sb = cpool.tile([P, P], dt_pt)
            ones_sb = cpool.tile([1, P], dt_x)
            nc.vector.memset(ones_sb[:], 1.0)
            ebias_sb = cpool.tile([P, 1], F32)
            nc.vector.memset(ebias_sb[:], exp_bias)

            def late_const_dmas():
                nc.sync.dma_start(wv_sb[:], wv_r)
                # w_o rows: chunk0 = h0,h1 dims (128 rows), chunk1 = h2 (64)
                nc.sync.dma_start(wo_sb[:, 0, :], wo_d.ap()[0:P, :])
                nc.sync.dma_start(wo_sb[0:64, 1, :], wo_d.ap()[P:D_LOCAL, :])
                nc.sync.dma_start(cm_sb[:], cm_d.ap())

            # ---- HAM warm-up ----
            # Dependency-free matmuls run during the initial input-DMA
            # wait, holding the PE busy past the 3.4us activity window so
            # the clock gate opens (1.2 -> 2.4 GHz) before real work lands
            # — and keep it open until the first q/k chunk has landed.
            wu = pp_sc.tile([P, 2 * NQ], F32, tag="psc")
            for i in range(110):
                # alternate PSUM banks so the dummies stream back-to-back
                base = (i % 2) * NQ
                nc.tensor.matmul(wu[0:64, base:base + 96],
                                 _mm(ones_sb[:, 0:64], mm),
                                 _mm(ones_sb[:, 0:96], mm),
                                 start=True, stop=True,
                                 skip_group_check=True)

            # ---- persistent activations ----
            qkT_sb = ppool.tile([P, 4, n], dt_acc)
            # v slab padded to 80 elements so the key-block stride is a
            # multiple of 16 bytes in fp8 (DoubleRow weight-AP constraint)
            v_sb = ppool.tile([P, TB, H_LOCAL, 80], dt_pt)
            outT_sb = ppool.tile([P, 2, n], dt_acc)
            nc.vector.memset(v_sb[:, :, :, 64:65], 1.0)

            # ---- q/k projection for one 512-token chunk ----
            def proj_dma(t):
                xq = xpool.tile([P, KO, NT], dt_x, tag="x")
                nc.sync.dma_start(xq[:], qt_r[:, :, t * NT:(t + 1) * NT])
                xk = xpool.tile([P, KO, NT], dt_x, tag="x")
                nc.sync.dma_start(xk[:], kt_r[:, :, t * NT:(t + 1) * NT])
                return xq, xk

            def proj_blk(t, xq, xk, blk):
                big = pp_sc.tile([P, 2 * NQ], F32, tag="psc")
                ps = big[:, 0:NQ]
                for ko in range(KO):
                    # blk1 contracts q2 against Q-input and k2 against
                    # K-input: split into two half-partition matmuls.
                    if blk == 1:
                        nc.tensor.matmul(
                            ps[0:64, 0:NT],
                            _mm(wqk_sb[:, ko, 128:192], mm),
                            _mm(xq[:, ko, :], mm),
                            start=(ko == 0), stop=(ko == KO - 1),
                            skip_group_check=True,
                        )
                        nc.tensor.matmul(
                            ps[64:128, 0:NT],
                            _mm(wqk_sb[:, ko, 192:256], mm),
                            _mm(xk[:, ko, :], mm),
                            start=(ko == 0), stop=(ko == KO - 1),
                            skip_group_check=True,
                        )
                    else:
                        x = xq if blk == 0 else xk
                        nc.tensor.matmul(
                            ps[:, 0:NT],
                            _mm(wqk_sb[:, ko, blk * 128:(blk + 1) * 128], mm),
                            _mm(x[:, ko, :], mm),
                            start=(ko == 0), stop=(ko == KO - 1),
                        )
                nc.vector.tensor_copy(
                    out=qkT_sb[:, blk, t * NT:(t + 1) * NT],
                    in_=ps[:, 0:NT],
                )
                if blk == 1:
                    # Partition-shifted copies so h2's scores matmul sees
                    # qT/kT at the same base — and at BOTH bases, so h2 can
                    # alternate row-groups and pair with whichever half is
                    # free:
                    #   blk3[0:64]   = k2 (from blk1[64:128])
                    #   blk3[64:128] = q2 (from blk1[0:64])
                    nc.gpsimd.dma_start(
                        qkT_sb[0:64, 3, t * NT:(t + 1) * NT],
                        qkT_sb[64:128, 1, t * NT:(t + 1) * NT],
                    )
                    nc.gpsimd.dma_start(
                        qkT_sb[64:128, 3, t * NT:(t + 1) * NT],
                        qkT_sb[0:64, 1, t * NT:(t + 1) * NT],
                    )

            # ---- v projection (token-major layout) for one 128-token block --
            def v_dma(tb):
                xv = xvpool.tile([P, KO, P], dt_x)
                nc.sync.dma_start(xv[:], vt_r[:, :, tb * P:(tb + 1) * P])
                return xv

            def v_blk(tb, xv):
                big = pp_sc.tile([P, 2 * NQ], F32, tag="psc")
                ps = big[:, 0:NQ]
                for ko in range(KO):
                    nc.tensor.matmul(
                        ps[:, 0:D_LOCAL],
                        _mm(xv[:, ko, :], mm),
                        _mm(wv_sb[:, ko, :], mm),
                        start=(ko == 0), stop=(ko == KO - 1),
                    )
                # one copy for all 3 heads: free pattern [3, 64]
                nc.vector.tensor_copy(
                    out=v_sb[:, tb, :, 0:64],
                    in_=ps[:, 0:D_LOCAL].rearrange("p (h d) -> p h d",
                                                   h=H_LOCAL),
                )

            # ---- causal attention, transposed-score flash style ----
            # Heads are interleaved so the PE runs two concurrent score
            # matmuls on disjoint row-groups: h0 lives at partitions 0-63,
            # h1 at 64-127, h2 alternates base per key-block (its qT/kT are
            # replicated at both bases in blk1/blk3).
            def h2_qk(kb):
                if kb % 2 == 0:
                    return (0, 1), (0, 3)   # q2 @ blk1[0:64], k2' @ blk3[0:64]
                return (64, 3), (64, 1)     # q2' @ blk3[64:128], k2 @ blk1[64:128]

            def qk_for(h, kb):
                if h == 2:
                    return h2_qk(kb)
                return q_loc[h], k_loc[h]

            def attn_chunk(j, filler, lowq, pre_pv=None):
                """filler/lowq: lists of thunks issuing ~1-3us of
                non-attention PE work each; one is drained after each
                key-block pair so the PE fills the slack of the ACT-bound
                inner loop instead of stalling ACT with a solid block
                between chunks.  filler holds work the NEXT chunk reads
                (force-drained by the caller); lowq holds deferrable
                output-projection work.  pre_pv (the previous chunk's
                normalization) is issued after pair 0's exps so its DVE
                chain hides under ACT work — it must precede pair 0's PV
                matmuls, which recycle the po tiles it reads."""
                po = [pp_out.tile([P, NQ], F32, tag=f"po{h}", name=f"po{h}")
                      for h in range(H_LOCAL)]
                nkb = KB_PER_Q * j + KB_PER_Q

                def keep_alive():
                    # tiny dependency-free matmul: fills PE wait slots so
                    # the HAM activity window never sees the PE idle and
                    # the 2.4 GHz clock gate stays open.
                    nc.tensor.matmul(ka_t.ap()[0:64, 0:96],
                                     _mm(ones_sb[:, 0:64], mm),
                                     _mm(ones_sb[:, 0:96], mm),
                                     start=True, stop=True,
                                     skip_group_check=True)

                def pv_mms(pts, ka, kb_, offa, offb):
                    if dr:
                        # One DoubleRow matmul per head contracts BOTH key
                        # blocks (K=256: 2 fp8 weights per PE cell) over
                        # the query range where both are causally valid;
                        # diagonal pairs get a small regular matmul for
                        # block a's extra strip [offa:offb).
                        def strips():
                            if offb > offa:
                                for h in range(H_LOCAL):
                                    nc.tensor.matmul(
                                        po[h][0:65, offa:offb],
                                        v_sb[:, ka, h, 0:65],
                                        pts[h][:, offa:offb],
                                        start=False, stop=False,
                                        skip_group_check=True,
                                    )

                        # group flags: start clears the whole bank (must
                        # come first), stop must be the group's last write.
                        if ka != 0:
                            strips()
                        for h in range(H_LOCAL):
                            vv = v_sb[:, ka:ka + 2, h, 0:65]
                            rh = pts[h][:].rearrange(
                                "p (two q) -> p two q", two=2)[:, :, offb:]
                            nc.tensor.matmul(
                                po[h][0:65, offb:], vv, rh,
                                start=(ka == 0), stop=(kb_ == nkb - 1),
                                perf_mode=mybir.MatmulPerfMode.DoubleRow,
                                skip_group_check=True,
                            )
                        if ka == 0:
                            strips()
                        return
                    # interleave heads so consecutive matmuls write
                    # different PSUM banks (no same-bank write pressure)
                    for h in range(H_LOCAL):
                        nc.tensor.matmul(
                            po[h][0:65, offa:],
                            _mm(v_sb[:, ka, h, 0:65], mm),
                            _mm(pts[h][:, offa:NQ], mm),
                            start=(ka == 0), stop=False,
                        )
                    for h in range(H_LOCAL):
                        nc.tensor.matmul(
                            po[h][0:65, offb:],
                            _mm(v_sb[:, kb_, h, 0:65], mm),
                            _mm(pts[h][:, NQ + offb:], mm),
                            start=False, stop=(kb_ == nkb - 1),
                        )

                prev = None
                for kb2 in range(0, nkb, 2):
                    ka, kb_ = kb2, kb2 + 1
                    offa = max(ka - KB_PER_Q * j, 0) * P
                    offb = max(kb_ - KB_PER_Q * j, 0) * P

                    def score_mm(h, kb, psc, base):
                        # Halves of the [128, 1024] two-bank PSUM tile.
                        # Half a starts at its causal cutoff offa; half b
                        # always covers the full bank so the fused exp
                        # below never reads never-written PSUM (the extra
                        # columns are fully-masked queries nothing
                        # consumes).
                        lo = offa if base == 0 else 0
                        (qp, qb), (kp, kbblk) = qk_for(h, kb)
                        nc.tensor.matmul(
                            psc[:, base + lo:base + NQ],
                            _mm(qkT_sb[kp:kp + 64, kbblk,
                                       kb * P:(kb + 1) * P], mm),
                            _mm(qkT_sb[qp:qp + 64, qb,
                                       j * NQ + lo:(j + 1) * NQ], mm),
                            start=True, stop=True, skip_group_check=True,
                        )

                    def exp_and_mask(psc):
                        pt = ptpool.tile([P, 2 * NQ], dt_pt, name="pt")
                        nc.scalar.activation(
                            pt[:, offa:], psc[:, offa:],
                            mybir.ActivationFunctionType.Exp,
                            bias=ebias_sb[:])
                        # causal-boundary masks
                        if ka >= KB_PER_Q * j:
                            nc.vector.tensor_mul(
                                out=pt[:, offa:offa + P],
                                in0=pt[:, offa:offa + P], in1=cm_sb[:])
                        if kb_ >= KB_PER_Q * j:
                            nc.vector.tensor_mul(
                                out=pt[:, NQ + offb:NQ + offb + P],
                                in0=pt[:, NQ + offb:NQ + offb + P],
                                in1=cm_sb[:])
                        return pt

                    # Issue order matters twice over: h0/h1 score matmuls
                    # run concurrently on disjoint PE row-groups, and h2's
                    # psc tile reuses h0's pool slot, so h2's matmuls must
                    # be issued after h0's exp (write-after-read on the
                    # same PSUM banks).
                    psc0 = pp_sc.tile([P, 2 * NQ], F32, tag="psc")
                    psc1 = pp_sc.tile([P, 2 * NQ], F32, tag="psc")
                    score_mm(0, ka, psc0, 0)
                    score_mm(1, ka, psc1, 0)
                    score_mm(0, kb_, psc0, NQ)
                    score_mm(1, kb_, psc1, NQ)
                    pts = [None, None, None]
                    pts[0] = exp_and_mask(psc0)
                    if kb2 == 0 and pre_pv is not None:
                        pre_pv()
                    # two-stage pipeline: the previous pair's PV matmuls
                    # are issued here — their ~2.3us of PE work covers
                    # exp-h0's latency, so the h2 score matmuls (which
                    # recycle h0's psc slot) start without stalling.
                    if prev is not None:
                        pv_mms(*prev)
                    if filler:
                        filler.pop(0)()
                    elif lowq:
                        lowq.pop(0)()
                    psc2 = pp_sc.tile([P, 2 * NQ], F32, tag="psc")
                    score_mm(2, ka, psc2, 0)
                    score_mm(2, kb_, psc2, NQ)
                    pts[1] = exp_and_mask(psc1)
                    pts[2] = exp_and_mask(psc2)
                    keep_alive()
                    prev = (pts, ka, kb_, offa, offb)
                pv_mms(*prev)
                if filler:
                    filler.pop(0)()
                elif lowq:
                    lowq.pop(0)()
                # DVE half of this chunk's normalization: reciprocals are
                # ready well before the next chunk's pre_pv needs them.
                r1s = []
                for h in range(H_LOCAL):
                    # the custom-DVE reciprocal misreads PSUM operands on
                    # real HW (sim allows it) — stage the denominator row
                    # in SBUF first.
                    dn = rpool.tile([1, NQ], F32, tag="dn", name="dn")
                    nc.vector.tensor_copy(out=dn[:], in_=po[h][64:65, :])
                    r1 = rpool.tile([1, NQ], F32, tag="r1", name="r1")
                    nc.vector.reciprocal_approx_fast(out=r1[:], in_=dn[:])
                    # bf16 copy so the broadcast matmul runs on the fast
                    # matmul path (fp32 moving operands are slower)
                    r1b = rpool.tile([1, NQ], dt_x, tag="r1b", name="r1b")
                    nc.vector.tensor_copy(out=r1b[:], in_=r1[:])
                    r1s.append(r1b)
                return po, r1s

            # ---- finish normalizing chunk j: broadcast 1/denom, multiply --
            def norm_chunk(j, po, r1s):
                for h in range(H_LOCAL):
                    # replicate 1/denom across 64 partitions on the (idle)
                    # GPSIMD engine; the multiply reads po from PSUM.
                    rr = rpool.tile([64, NQ], dt_x, tag="rrsb", name="rrsb")
                    nc.gpsimd.partition_broadcast(rr[:], r1s[h][:])
                    if h == 1:
                        # h1 lives at partitions 64-127 of outT blk0; DVE
                        # lanes are partition-locked, so write a temp at
                        # base 0 and DMA partition-shift it up.
                        ot = otpool.tile([64, NQ], dt_acc, name="ot")
                        nc.vector.tensor_mul(out=ot[:], in0=po[h][0:64, :],
                                             in1=rr[:])
                        nc.sync.dma_start(
                            outT_sb[64:128, 0, j * NQ:(j + 1) * NQ], ot[:])
                    else:
                        dst = outT_sb[0:64, 0 if h == 0 else 1,
                                      j * NQ:(j + 1) * NQ]
                        nc.vector.tensor_mul(out=dst, in0=po[h][0:64, :],
                                             in1=rr[:])

            # ---- output projection: two 128-outdim blocks per call, for
            # the 512-token slice of chunk j.  yT[od, t] = wo.T @ outT.
            def outproj_ob(j, ob2):
                big = pp_sc.tile([P, 2 * NQ], F32, tag="psc")
                # k-chunk-major: consecutive matmuls alternate PSUM banks
                # and reuse the same stationary operand
                for half, ob in ((0, 2 * ob2), (NQ, 2 * ob2 + 1)):
                    nc.tensor.matmul(
                        big[:, half:half + NQ],
                        _mm(wo_sb[:, 0, ob * P:(ob + 1) * P], mm),
                        _mm(outT_sb[:, 0, j * NQ:(j + 1) * NQ], mm),
                        start=True, stop=False, skip_group_check=True,
                    )
                for half, ob in ((0, 2 * ob2), (NQ, 2 * ob2 + 1)):
                    nc.tensor.matmul(
                        big[:, half:half + NQ],
                        _mm(wo_sb[0:64, 1, ob * P:(ob + 1) * P], mm),
                        _mm(outT_sb[0:64, 1, j * NQ:(j + 1) * NQ], mm),
                        start=False, stop=True, skip_group_check=True,
                    )
                ysb = ypool.tile([P, 2, NQ], F32)
                nc.vector.tensor_copy(
                    out=ysb[:],
                    in_=big[:].rearrange("p (two q) -> p two q", two=2),
                )
                nc.sync.dma_start(
                    y_d.ap()[ob2 * 2 * P:(ob2 + 1) * 2 * P,
                             j * NQ:(j + 1) * NQ]
                    .rearrange("(two r) t -> r two t", two=2),
                    ysb[:],
                )

            # ---- software-pipelined schedule ----
            # prologue: first q/k chunk (blk1 last — h0/h1 scores only need
            # blk0+blk2) and the first 4 v blocks.
            xq0, xk0 = proj_dma(0)
            late_const_dmas()
            xvs = [v_dma(tb) for tb in range(min(KB_PER_Q, TB))]
            for blk in (0, 2, 1):
                proj_blk(0, xq0, xk0, blk)
            # v blocks 0-1 feed chunk 0's first pair and must precede it;
            # 2-3 (used by its second pair) slot in as its first filler.
            for tb, xv in list(enumerate(xvs))[:2]:
                v_blk(tb, xv)
            lowq = []
            po_prev = None
            for j in range(QCH):
                # stage next round's inputs and build its filler list
                filler = []
                if j == 0:
                    def _v23(rest=list(enumerate(xvs))[2:]):
                        for tb, xv in rest:
                            v_blk(tb, xv)
                    filler.append(_v23)
                if j + 1 < TCH:
                    xq, xk = proj_dma(j + 1)
                    for blk in (0, 2, 1):
                        filler.append(
                            lambda t=j + 1, a=xq, b=xk, bl=blk:
                            proj_blk(t, a, b, bl))
                for tb in range(KB_PER_Q * (j + 1),
                                min(KB_PER_Q * (j + 2), TB)):
                    xv = v_dma(tb)
                    filler.append(lambda t=tb, x=xv: v_blk(t, x))
                if j > 0:
                    for ob2 in range(3):
                        lowq.append(lambda jj=j - 1, o=ob2:
                                    outproj_ob(jj, o))
                pre_pv = None
                if po_prev is not None:
                    pre_pv = (lambda jj=j - 1, pp=po_prev:
                              norm_chunk(jj, *pp))
                po_prev = attn_chunk(j, filler, lowq, pre_pv)
                # next chunk reads these — they must be issued before it
                for fn in filler:
                    fn()
            norm_chunk(QCH - 1, *po_prev)
            for fn in lowq:
                fn()
            for ob2 in range(3):
                outproj_ob(QCH - 1, ob2)

    nc.compile()
    return nc


def make_causal_mask_np(dt=np.float32):
    """[128, 128] lower-left keep mask: m[p, f] = 1.0 iff f >= p."""
    f = np.arange(P)[None, :]
    p = np.arange(P)[:, None]
    return (f >= p).astype(np.float32).astype(dt)


def prep_core_inputs(Q, K, V, w_q, w_k, w_v, w_o, core, n=N_TOKENS,
                     np_x=ml_dtypes.bfloat16, np_pt=ml_dtypes.bfloat16):
    """Host-side sharding/layout prep for one core. All fp32 numpy in."""
    b = core // 4
    g = core % 4
    hs = g * D_LOCAL
    scale = 1.0 / np.sqrt(D_K)
    qt = np.ascontiguousarray(Q[b].T).astype(np_x)
    kt = np.ascontiguousarray(K[b].T).astype(np_x)
    vt = np.ascontiguousarray(V[b].T).astype(np_x)
    wql = w_q[hs:hs + D_LOCAL] * scale
    wkl = w_k[hs:hs + D_LOCAL]
    # column order [q0 q1 | q2 k2 | k0 k1] (see build_nc)
    wqk = np.ascontiguousarray(
        np.concatenate([wql[0:128], wql[128:192], wkl[128:192], wkl[0:128]],
                       axis=0).T
    ).astype(np_x)
    wv = np.ascontiguousarray(w_v[hs:hs + D_LOCAL].T).astype(np_x)
    wo = np.ascontiguousarray(w_o[:, hs:hs + D_LOCAL].T).astype(np_x)
    cm = make_causal_mask_np(np_pt)
    return {"qt": qt, "kt": kt, "vt": vt, "wqk": wqk, "wv": wv, "wo": wo,
            "cmask": cm}


_NC_CACHE = {}


def _get_nc(key, **kw):
    if key not in _NC_CACHE:
        _NC_CACHE[key] = build_nc(**kw)
    return _NC_CACHE[key]


KCFG = {"mm": "bf16", "dt_x": BF16, "dt_pt": BF16, "dt_acc": BF16,
        "np_x": ml_dtypes.bfloat16, "np_pt": ml_dtypes.bfloat16}


def kernel(Q, K, V, w_q, w_k, w_v, w_o):
    Q = np.asarray(Q, dtype=np.float32)
    K = np.asarray(K, dtype=np.float32)
    V = np.asarray(V, dtype=np.float32)
    w_q = np.asarray(w_q, dtype=np.float32)
    w_k = np.asarray(w_k, dtype=np.float32)
    w_v = np.asarray(w_v, dtype=np.float32)
    w_o = np.asarray(w_o, dtype=np.float32)

    nc = _get_nc((KCFG["mm"], str(KCFG["dt_x"])),
                 n=N_TOKENS, mm=KCFG["mm"], dt_x=KCFG["dt_x"],
                 dt_pt=KCFG["dt_pt"], dt_acc=KCFG["dt_acc"])
    in_maps = [
        prep_core_inputs(Q, K, V, w_q, w_k, w_v, w_o, c,
                         np_x=KCFG["np_x"], np_pt=KCFG["np_pt"])
        for c in range(N_CORES)
    ]
    res = bass_utils.run_bass_kernel_spmd(nc, in_maps,
                                          core_ids=list(range(N_CORES)))
    out = np.zeros((B, N_TOKENS, D_MODEL), dtype=np.float32)
    for c in range(N_CORES):
        out[c // 4] += res.results[c]["y"].T
    return out


# revision 80
# speedup vs baseline: 1.1700x; 1.1700x over previous
"""Trainium2 Bass kernel for causal MHA (b=2, n=4096, d_model=768, 12 heads).

Sharding: 8 cores = 2 batches x 4 head-groups (3 heads each).
Each core:
  - receives its batch's Q/K/V pre-transposed ([768, n], d_model on rows)
    plus its head-group's weight slices (also pre-transposed on host).
  - projects qT/kT ([64, n] per head, head dim on partitions) and
    v ([n, 64] per head, tokens on partitions) on-chip.
  - computes scoresT[k, q] = kT^T @ qT for PAIRS of 128-key blocks into a
    single [128, 1024] two-bank PSUM tile, exponentiates the pair with ONE
    activation instruction (halves the per-instruction ACT overhead),
    masks causal-boundary blocks with a precomputed 0/1 mask, and
    accumulates outT_aug[65, q] += [v | ones]^T @ P in PSUM.  Row 64 is
    the softmax denominator.
  - normalizes per query-chunk: 1/denom via the fast custom-DVE
    reciprocal, replicated across 64 partitions with a K=1 PE matmul
    (ones[1,64]^T @ r1[1,512] -> PSUM), then one tensor_mul into outT.
    No DRAM round-trips.
  - applies the output projection with its w_o row-slice; host sums the
    4 partial outputs per batch (row-parallel linear unshard).

The phases are software-pipelined: projection chunk t+1, v-projection
blocks, and the output projection for chunk j-1 are issued between
attention chunks so the PE fills the slack of the ACT-bound attention
inner loop and the HAM clock gate stays warm (2.4 GHz).

Weight-column host layout packs the six 64-wide q/k heads into three full
128-row M-blocks ([q0;q1], [q2;k2], [k0;k1]); k2/q2 are then DMA-copied
into a fourth block so every head's scores matmul sees its qT and kT at
the same partition base (a matmul constraint), with h2's operands at BOTH
bases so it can pair with whichever PE row-group is free.
"""

import sys

for _p in ("/opt/trn_rl_repo",):
    if _p not in sys.path:
        sys.path.insert(0, _p)

import numpy as np
import ml_dtypes

import concourse.bass as bass  # noqa: F401  (registers engine classes)
import concourse.tile as tile
from concourse import bacc, mybir
import concourse.bass_utils as bass_utils

P = 128
D_MODEL = 768
KO = D_MODEL // P  # 6 contraction chunks of 128
N_HEADS = 12
D_K = 64
N_CORES = 8
H_LOCAL = 3  # heads per core
D_LOCAL = H_LOCAL * D_K  # 192
B = 2
N_TOKENS = 4096
NQ = 512  # query-chunk size (one PSUM bank of fp32)
NT = 512  # token chunk for q/k projection

F32 = mybir.dt.float32
BF16 = mybir.dt.bfloat16
F32R = mybir.dt.float32r
F8 = mybir.dt.float8e4  # TRN e4m3, max normal 240

# exp(s + ln(1/8)): keeps exp outputs < 240/8 so they fit fp8 e4m3; the
# constant cancels exactly in the softmax division.
EXP_BIAS_FP8 = float(np.log(1.0 / 8.0))

DEBUG = False


def _mm(ap, flavor):
    """View an fp32 AP as the matmul input dtype."""
    if flavor == "f32r":
        return ap.bitcast(F32R)
    return ap


def build_nc(n=N_TOKENS, mm="bf16", dt_x=BF16, dt_pt=BF16, dt_acc=BF16):
    assert n % NQ == 0 and n % NT == 0 and n % P == 0
    # fp8 pt enables DoubleRow PV matmuls (key-block pair fused, K=256)
    dr = dt_pt == F8
    exp_bias = EXP_BIAS_FP8 if dr else 0.0
    nc = bacc.Bacc("TRN2", target_bir_lowering=False, debug=False,
                   num_devices=N_CORES)

    qt_d = nc.dram_tensor("qt", [D_MODEL, n], dt_x, kind="ExternalInput")
    kt_d = nc.dram_tensor("kt", [D_MODEL, n], dt_x, kind="ExternalInput")
    vt_d = nc.dram_tensor("vt", [D_MODEL, n], dt_x, kind="ExternalInput")
    wqk_d = nc.dram_tensor("wqk", [D_MODEL, 2 * D_LOCAL], dt_x,
                           kind="ExternalInput")
    wv_d = nc.dram_tensor("wv", [D_MODEL, D_LOCAL], dt_x, kind="ExternalInput")
    wo_d = nc.dram_tensor("wo", [D_LOCAL, D_MODEL], dt_x, kind="ExternalInput")
    cm_d = nc.dram_tensor("cmask", [P, P], dt_pt, kind="ExternalInput")
    # transposed output [d_model, n]: the host transposes after gather —
    # this lets the output projection keep w_o stationary (12 weight loads
    # total) and stream outT as the moving operand.
    y_d = nc.dram_tensor("y", [D_MODEL, n], F32, kind="ExternalOutput")
    if DEBUG:
        dbg_d = nc.dram_tensor("dbg", [3 * (n // NQ) * H_LOCAL, NQ], F32,
                               kind="ExternalOutput")

    qt_r = qt_d.ap().rearrange("(ko ki) t -> ki ko t", ki=P)
    kt_r = kt_d.ap().rearrange("(ko ki) t -> ki ko t", ki=P)
    vt_r = vt_d.ap().rearrange("(ko ki) t -> ki ko t", ki=P)
    wqk_r = wqk_d.ap().rearrange("(ko ki) m -> ki ko m", ki=P)
    wv_r = wv_d.ap().rearrange("(ko ki) m -> ki ko m", ki=P)

    TCH = n // NT       # q/k projection token chunks
    TB = n // P         # 128-token blocks
    QCH = n // NQ       # query chunks
    KB_PER_Q = NQ // P  # key blocks per query chunk (4)

    # Host weight-column order: [q0 q1 | q2 k2 | k0 k1] -> 3 full M-blocks.
    # qkT_sb blk3 holds DMA-shifted copies: [0:64]=k2, [64:128]=q2.
    q_loc = {0: (0, 0), 1: (64, 0), 2: (0, 1)}
    k_loc = {0: (0, 2), 1: (64, 2), 2: (0, 3)}

    # one raw PSUM bank for HAM keep-alive dummy matmuls (outside the
    # tile pools; only the PE ever touches it, in queue order)
    ka_t = nc.alloc_psum_tensor("keepalive", [64, 96], F32)

    with tile.TileContext(nc) as tc:
        with tc.tile_pool(name="const", bufs=1) as cpool, \
             tc.tile_pool(name="persist", bufs=1) as ppool, \
             tc.tile_pool(name="xqk", bufs=4) as xpool, \
             tc.tile_pool(name="xv", bufs=6) as xvpool, \
             tc.tile_pool(name="pt", bufs=9) as ptpool, \
             tc.tile_pool(name="ysb", bufs=2) as ypool, \
             tc.tile_pool(name="rcp", bufs=4) as rpool, \
             tc.tile_pool(name="ot", bufs=2) as otpool, \
             tc.tile_pool(name="pp_sc", bufs=2, space="PSUM") as pp_sc, \
             tc.tile_pool(name="pp_out", bufs=1, space="PSUM") as pp_out:

            # ---- constants ----
            # DMA issue order = sync-queue order: the first projection
            # needs wqk and the first q/k chunk, so those go first; wv/wo/
            # cm follow behind them.
            wqk_sb = cpool.tile([P, KO, 2 * D_LOCAL], dt_x)
            nc.sync.dma_start(wqk_sb[:], wqk_r)
            wv_sb = cpool.tile([P, KO, D_LOCAL], dt_x)
            wo_sb = cpool.tile([P, 2, D_MODEL], dt_x)
            cm_sb = cpool.tile([P, P], dt_pt)
            ones_sb = cpool.tile([1, P], dt_x)
            nc.vector.memset(ones_sb[:], 1.0)
            ebias_sb = cpool.tile([P, 1], F32)
            nc.vector.memset(ebias_sb[:], exp_bias)

            def late_const_dmas():
                nc.sync.dma_start(wv_sb[:], wv_r)
                # w_o rows: chunk0 = h0,h1 dims (128 rows), chunk1 = h2 (64)
                nc.sync.dma_start(wo_sb[:, 0, :], wo_d.ap()[0:P, :])
                nc.sync.dma_start(wo_sb[0:64, 1, :], wo_d.ap()[P:D_LOCAL, :])
                nc.sync.dma_start(cm_sb[:], cm_d.ap())

            # ---- HAM warm-up ----
            # Dependency-free matmuls run during the initial input-DMA
            # wait, holding the PE busy past the 3.4us activity window so
            # the clock gate opens (1.2 -> 2.4 GHz) before real work lands
            # — and keep it open until the first q/k chunk has landed.
            wu = pp_sc.tile([P, 2 * NQ], F32, tag="psc")
            for i in range(110):
                # alternate PSUM banks so the dummies stream back-to-back
                base = (i % 2) * NQ
                nc.tensor.matmul(wu[0:64, base:base + 96],
                                 _mm(ones_sb[:, 0:64], mm),
                                 _mm(ones_sb[:, 0:96], mm),
                                 start=True, stop=True,
                                 skip_group_check=True)

            # ---- persistent activations ----
            qkT_sb = ppool.tile([P, 4, n], dt_acc)
            # v slab padded to 80 elements so the key-block stride is a
            # multiple of 16 bytes in fp8 (DoubleRow weight-AP constraint)
            v_sb = ppool.tile([P, TB, H_LOCAL, 80], dt_pt)
            outT_sb = ppool.tile([P, 2, n], dt_acc)
            nc.vector.memset(v_sb[:, :, :, 64:65], 1.0)

            # ---- q/k projection for one 512-token chunk ----
            def proj_dma(t):
                xq = xpool.tile([P, KO, NT], dt_x, tag="x")
                nc.sync.dma_start(xq[:], qt_r[:, :, t * NT:(t + 1) * NT])
                xk = xpool.tile([P, KO, NT], dt_x, tag="x")
                nc.sync.dma_start(xk[:], kt_r[:, :, t * NT:(t + 1) * NT])
                return xq, xk

            def proj_blk(t, xq, xk, blk):
                big = pp_sc.tile([P, 2 * NQ], F32, tag="psc")
                ps = big[:, 0:NQ]
                for ko in range(KO):
                    # blk1 contracts q2 against Q-input and k2 against
                    # K-input: split into two half-partition matmuls.
                    if blk == 1:
                        nc.tensor.matmul(
                            ps[0:64, 0:NT],
                            _mm(wqk_sb[:, ko, 128:192], mm),
                            _mm(xq[:, ko, :], mm),
                            start=(ko == 0), stop=(ko == KO - 1),
                            skip_group_check=True,
                        )
                        nc.tensor.matmul(
                            ps[64:128, 0:NT],
                            _mm(wqk_sb[:, ko, 192:256], mm),
                            _mm(xk[:, ko, :], mm),
                            start=(ko == 0), stop=(ko == KO - 1),
                            skip_group_check=True,
                        )
                    else:
                        x = xq if blk == 0 else xk
                        nc.tensor.matmul(
                            ps[:, 0:NT],
                            _mm(wqk_sb[:, ko, blk * 128:(blk + 1) * 128], mm),
                            _mm(x[:, ko, :], mm),
                            start=(ko == 0), stop=(ko == KO - 1),
                        )
                nc.vector.tensor_copy(
                    out=qkT_sb[:, blk, t * NT:(t + 1) * NT],
                    in_=ps[:, 0:NT],
                )
                if blk == 1:
                    # Partition-shifted copies so h2's scores matmul sees
                    # qT/kT at the same base — and at BOTH bases, so h2 can
                    # alternate row-groups and pair with whichever half is
                    # free:
                    #   blk3[0:64]   = k2 (from blk1[64:128])
                    #   blk3[64:128] = q2 (from blk1[0:64])
                    nc.gpsimd.dma_start(
                        qkT_sb[0:64, 3, t * NT:(t + 1) * NT],
                        qkT_sb[64:128, 1, t * NT:(t + 1) * NT],
                    )
                    nc.gpsimd.dma_start(
                        qkT_sb[64:128, 3, t * NT:(t + 1) * NT],
                        qkT_sb[0:64, 1, t * NT:(t + 1) * NT],
                    )

            # ---- v projection (token-major layout) for one 128-token block --
            def v_dma(tb):
                xv = xvpool.tile([P, KO, P], dt_x)
                nc.sync.dma_start(xv[:], vt_r[:, :, tb * P:(tb + 1) * P])
                return xv

            def v_blk(tb, xv):
                big = pp_sc.tile([P, 2 * NQ], F32, tag="psc")
                ps = big[:, 0:NQ]
                for ko in range(KO):
                    nc.tensor.matmul(
                        ps[:, 0:D_LOCAL],
                        _mm(xv[:, ko, :], mm),
                        _mm(wv_sb[:, ko, :], mm),
                        start=(ko == 0), stop=(ko == KO - 1),
                    )
                # one copy for all 3 heads: free pattern [3, 64]
                nc.vector.tensor_copy(
                    out=v_sb[:, tb, :, 0:64],
                    in_=ps[:, 0:D_LOCAL].rearrange("p (h d) -> p h d",
                                                   h=H_LOCAL),
                )

            # ---- causal attention, transposed-score flash style ----
            # Heads are interleaved so the PE runs two concurrent score
            # matmuls on disjoint row-groups: h0 lives at partitions 0-63,
            # h1 at 64-127, h2 alternates base per key-block (its qT/kT are
            # replicated at both bases in blk1/blk3).
            def h2_qk(kb):
                if kb % 2 == 0:
                    return (0, 1), (0, 3)   # q2 @ blk1[0:64], k2' @ blk3[0:64]
                return (64, 3), (64, 1)     # q2' @ blk3[64:128], k2 @ blk1[64:128]

            def qk_for(h, kb):
                if h == 2:
                    return h2_qk(kb)
                return q_loc[h], k_loc[h]

            def attn_chunk(j, filler, lowq, pre_pv=None):
                """filler/lowq: lists of thunks issuing ~1-3us of
                non-attention PE work each; one is drained after each
                key-block pair so the PE fills the slack of the ACT-bound
                inner loop instead of stalling ACT with a solid block
                between chunks.  filler holds work the NEXT chunk reads
                (force-drained by the caller); lowq holds deferrable
                output-projection work.  pre_pv (the previous chunk's
                normalization) is issued after pair 0's exps so its DVE
                chain hides under ACT work — it must precede pair 0's PV
                matmuls, which recycle the po tiles it reads."""
                po = [pp_out.tile([P, NQ], F32, tag=f"po{h}", name=f"po{h}")
                      for h in range(H_LOCAL)]
                nkb = KB_PER_Q * j + KB_PER_Q

                def keep_alive():
                    # tiny dependency-free matmul: fills PE wait slots so
                    # the HAM activity window never sees the PE idle and
                    # the 2.4 GHz clock gate stays open.
                    nc.tensor.matmul(ka_t.ap()[0:64, 0:96],
                                     _mm(ones_sb[:, 0:64], mm),
                                     _mm(ones_sb[:, 0:96], mm),
                                     start=True, stop=True,
                                     skip_group_check=True)

                def pv_mms(pts, ka, kb_, offa, offb):
                    if dr:
                        # One DoubleRow matmul per head contracts BOTH key
                        # blocks (K=256: 2 fp8 weights per PE cell) over
                        # the query range where both are causally valid;
                        # diagonal pairs get a small regular matmul for
                        # block a's extra strip [offa:offb).
                        def strips():
                            if offb > offa:
                                for h in range(H_LOCAL):
                                    nc.tensor.matmul(
                                        po[h][0:65, offa:offb],
                                        v_sb[:, ka, h, 0:65],
                                        pts[h][:, offa:offb],
                                        start=False, stop=False,
                                        skip_group_check=True,
                                    )

                        # group flags: start clears the whole bank (must
                        # come first), stop must be the group's last write.
                        if ka != 0:
                            strips()
                        for h in range(H_LOCAL):
                            vv = v_sb[:, ka:ka + 2, h, 0:65]
                            rh = pts[h][:].rearrange(
                                "p (two q) -> p two q", two=2)[:, :, offb:]
                            nc.tensor.matmul(
                                po[h][0:65, offb:], vv, rh,
                                start=(ka == 0), stop=(kb_ == nkb - 1),
                                perf_mode=mybir.MatmulPerfMode.DoubleRow,
                                skip_group_check=True,
                            )
                        if ka == 0:
                            strips()
                        return
                    # interleave heads so consecutive matmuls write
                    # different PSUM banks (no same-bank write pressure)
                    for h in range(H_LOCAL):
                        nc.tensor.matmul(
                            po[h][0:65, offa:],
                            _mm(v_sb[:, ka, h, 0:65], mm),
                            _mm(pts[h][:, offa:NQ], mm),
                            start=(ka == 0), stop=False,
                        )
                    for h in range(H_LOCAL):
                        nc.tensor.matmul(
                            po[h][0:65, offb:],
                            _mm(v_sb[:, kb_, h, 0:65], mm),
                            _mm(pts[h][:, NQ + offb:], mm),
                            start=False, stop=(kb_ == nkb - 1),
                        )

                prev = None
                for kb2 in range(0, nkb, 2):
                    ka, kb_ = kb2, kb2 + 1
                    offa = max(ka - KB_PER_Q * j, 0) * P
                    offb = max(kb_ - KB_PER_Q * j, 0) * P

                    def score_mm(h, kb, psc, base):
                        # Halves of the [128, 1024] two-bank PSUM tile.
                        # Half a starts at its causal cutoff offa; half b
                        # always covers the full bank so the fused exp
                        # below never reads never-written PSUM (the extra
                        # columns are fully-masked queries nothing
                        # consumes).
                        lo = offa if base == 0 else 0
                        (qp, qb), (kp, kbblk) = qk_for(h, kb)
                        nc.tensor.matmul(
                            psc[:, base + lo:base + NQ],
                            _mm(qkT_sb[kp:kp + 64, kbblk,
                                       kb * P:(kb + 1) * P], mm),
                            _mm(qkT_sb[qp:qp + 64, qb,
                                       j * NQ + lo:(j + 1) * NQ], mm),
                            start=True, stop=True, skip_group_check=True,
                        )

                    def exp_and_mask(psc):
                        pt = ptpool.tile([P, 2 * NQ], dt_pt, name="pt")
                        nc.scalar.activation(
                            pt[:, offa:], psc[:, offa:],
                            mybir.ActivationFunctionType.Exp,
                            bias=ebias_sb[:])
                        # causal-boundary masks
                        if ka >= KB_PER_Q * j:
                            nc.vector.tensor_mul(
                                out=pt[:, offa:offa + P],
                                in0=pt[:, offa:offa + P], in1=cm_sb[:])
                        if kb_ >= KB_PER_Q * j:
                            nc.vector.tensor_mul(
                                out=pt[:, NQ + offb:NQ + offb + P],
                                in0=pt[:, NQ + offb:NQ + offb + P],
                                in1=cm_sb[:])
                        return pt

                    # Issue order matters twice over: h0/h1 score matmuls
                    # run concurrently on disjoint PE row-groups, and h2's
                    # psc tile reuses h0's pool slot, so h2's matmuls must
                    # be issued after h0's exp (write-after-read on the
                    # same PSUM banks).
                    psc0 = pp_sc.tile([P, 2 * NQ], F32, tag="psc")
                    psc1 = pp_sc.tile([P, 2 * NQ], F32, tag="psc")
                    score_mm(0, ka, psc0, 0)
                    score_mm(1, ka, psc1, 0)
                    score_mm(0, kb_, psc0, NQ)
                    score_mm(1, kb_, psc1, NQ)
                    pts = [None, None, None]
                    pts[0] = exp_and_mask(psc0)
                    psc2 = pp_sc.tile([P, 2 * NQ], F32, tag="psc")
                    score_mm(2, ka, psc2, 0)
                    score_mm(2, kb_, psc2, NQ)
                    pts[1] = exp_and_mask(psc1)
                    pts[2] = exp_and_mask(psc2)
                    keep_alive()
                    if kb2 == 0 and pre_pv is not None:
                        pre_pv()
                    # two-stage pipeline: the previous pair's PV matmuls
                    # are issued only now, so they never head-of-line
                    # block the PE waiting on their exps.
                    if prev is not None:
                        pv_mms(*prev)
                        if filler:
                            filler.pop(0)()
                        elif lowq:
                            lowq.pop(0)()
                    prev = (pts, ka, kb_, offa, offb)
                pv_mms(*prev)
                if filler:
                    filler.pop(0)()
                elif lowq:
                    lowq.pop(0)()
                # DVE half of this chunk's normalization: reciprocals are
                # ready well before the next chunk's pre_pv needs them.
                r1s = []
                for h in range(H_LOCAL):
                    # the custom-DVE reciprocal misreads PSUM operands on
                    # real HW (sim allows it) — stage the denominator row
                    # in SBUF first.
                    dn = rpool.tile([1, NQ], F32, tag="dn", name="dn")
                    nc.vector.tensor_copy(out=dn[:], in_=po[h][64:65, :])
                    r1 = rpool.tile([1, NQ], F32, tag="r1", name="r1")
                    nc.vector.reciprocal_approx_fast(out=r1[:], in_=dn[:])
                    # bf16 copy so the broadcast matmul runs on the fast
                    # matmul path (fp32 moving operands are slower)
                    r1b = rpool.tile([1, NQ], dt_x, tag="r1b", name="r1b")
                    nc.vector.tensor_copy(out=r1b[:], in_=r1[:])
                    r1s.append(r1b)
                return po, r1s

            # ---- finish normalizing chunk j: broadcast 1/denom, multiply --
            def norm_chunk(j, po, r1s):
                for h in range(H_LOCAL):
                    # replicate 1/denom across 64 partitions on the (idle)
                    # GPSIMD engine; the multiply reads po from PSUM.
                    rr = rpool.tile([64, NQ], dt_x, tag="rrsb", name="rrsb")
                    nc.gpsimd.partition_broadcast(rr[:], r1s[h][:])
                    if h == 1:
                        # h1 lives at partitions 64-127 of outT blk0; DVE
                        # lanes are partition-locked, so write a temp at
                        # base 0 and DMA partition-shift it up.
                        ot = otpool.tile([64, NQ], dt_acc, name="ot")
                        nc.vector.tensor_mul(out=ot[:], in0=po[h][0:64, :],
                                             in1=rr[:])
                        nc.gpsimd.dma_start(
                            outT_sb[64:128, 0, j * NQ:(j + 1) * NQ], ot[:])
                    else:
                        dst = outT_sb[0:64, 0 if h == 0 else 1,
                                      j * NQ:(j + 1) * NQ]
                        nc.vector.tensor_mul(out=dst, in0=po[h][0:64, :],
                                             in1=rr[:])

            # ---- output projection: two 128-outdim blocks per call, for
            # the 512-token slice of chunk j.  yT[od, t] = wo.T @ outT.
            def outproj_ob(j, ob2):
                big = pp_sc.tile([P, 2 * NQ], F32, tag="psc")
                # k-chunk-major: consecutive matmuls alternate PSUM banks
                # and reuse the same stationary operand
                for half, ob in ((0, 2 * ob2), (NQ, 2 * ob2 + 1)):
                    nc.tensor.matmul(
                        big[:, half:half + NQ],
                        _mm(wo_sb[:, 0, ob * P:(ob + 1) * P], mm),
                        _mm(outT_sb[:, 0, j * NQ:(j + 1) * NQ], mm),
                        start=True, stop=False, skip_group_check=True,
                    )
                for half, ob in ((0, 2 * ob2), (NQ, 2 * ob2 + 1)):
                    nc.tensor.matmul(
                        big[:, half:half + NQ],
                        _mm(wo_sb[0:64, 1, ob * P:(ob + 1) * P], mm),
                        _mm(outT_sb[0:64, 1, j * NQ:(j + 1) * NQ], mm),
                        start=False, stop=True, skip_group_check=True,
                    )
                ysb = ypool.tile([P, 2, NQ], F32)
                nc.vector.tensor_copy(
                    out=ysb[:],
                    in_=big[:].rearrange("p (two q) -> p two q", two=2),
                )
                nc.sync.dma_start(
                    y_d.ap()[ob2 * 2 * P:(ob2 + 1) * 2 * P,
                             j * NQ:(j + 1) * NQ]
                    .rearrange("(two r) t -> r two t", two=2),
                    ysb[:],
                )

            # ---- software-pipelined schedule ----
            # prologue: first q/k chunk (blk1 last — h0/h1 scores only need
            # blk0+blk2) and the first 4 v blocks.
            xq0, xk0 = proj_dma(0)
            late_const_dmas()
            xvs = [v_dma(tb) for tb in range(min(KB_PER_Q, TB))]
            for blk in (0, 2, 1):
                proj_blk(0, xq0, xk0, blk)
            # v blocks 0-1 feed chunk 0's first pair and must precede it;
            # 2-3 (used by its second pair) slot in as its first filler.
            for tb, xv in list(enumerate(xvs))[:2]:
                v_blk(tb, xv)
            lowq = []
            po_prev = None
            for j in range(QCH):
                # stage next round's inputs and build its filler list
                filler = []
                if j == 0:
                    def _v23(rest=list(enumerate(xvs))[2:]):
                        for tb, xv in rest:
                            v_blk(tb, xv)
                    filler.append(_v23)
                if j + 1 < TCH:
                    xq, xk = proj_dma(j + 1)
                    for blk in (0, 2, 1):
                        filler.append(
                            lambda t=j + 1, a=xq, b=xk, bl=blk:
                            proj_blk(t, a, b, bl))
                for tb in range(KB_PER_Q * (j + 1),
                                min(KB_PER_Q * (j + 2), TB)):
                    xv = v_dma(tb)
                    filler.append(lambda t=tb, x=xv: v_blk(t, x))
                if j > 0:
                    for ob2 in range(3):
                        lowq.append(lambda jj=j - 1, o=ob2:
                                    outproj_ob(jj, o))
                pre_pv = None
                if po_prev is not None:
                    pre_pv = (lambda jj=j - 1, pp=po_prev:
                              norm_chunk(jj, *pp))
                po_prev = attn_chunk(j, filler, lowq, pre_pv)
                # next chunk reads these — they must be issued before it
                for fn in filler:
                    fn()
            norm_chunk(QCH - 1, *po_prev)
            for fn in lowq:
                fn()
            for ob2 in range(3):
                outproj_ob(QCH - 1, ob2)

    nc.compile()
    return nc


def make_causal_mask_np(dt=np.float32):
    """[128, 128] lower-left keep mask: m[p, f] = 1.0 iff f >= p."""
    f = np.arange(P)[None, :]
    p = np.arange(P)[:, None]
    return (f >= p).astype(np.float32).astype(dt)


def prep_core_inputs(Q, K, V, w_q, w_k, w_v, w_o, core, n=N_TOKENS,
                     np_x=ml_dtypes.bfloat16, np_pt=ml_dtypes.bfloat16):
    """Host-side sharding/layout prep for one core. All fp32 numpy in."""
    b = core // 4
    g = core % 4
    hs = g * D_LOCAL
    scale = 1.0 / np.sqrt(D_K)
    qt = np.ascontiguousarray(Q[b].T).astype(np_x)
    kt = np.ascontiguousarray(K[b].T).astype(np_x)
    vt = np.ascontiguousarray(V[b].T).astype(np_x)
    wql = w_q[hs:hs + D_LOCAL] * scale
    wkl = w_k[hs:hs + D_LOCAL]
    # column order [q0 q1 | q2 k2 | k0 k1] (see build_nc)
    wqk = np.ascontiguousarray(
        np.concatenate([wql[0:128], wql[128:192], wkl[128:192], wkl[0:128]],
                       axis=0).T
    ).astype(np_x)
    wv = np.ascontiguousarray(w_v[hs:hs + D_LOCAL].T).astype(np_x)
    wo = np.ascontiguousarray(w_o[:, hs:hs + D_LOCAL].T).astype(np_x)
    cm = make_causal_mask_np(np_pt)
    return {"qt": qt, "kt": kt, "vt": vt, "wqk": wqk, "wv": wv, "wo": wo,
            "cmask": cm}


_NC_CACHE = {}


def _get_nc(key, **kw):
    if key not in _NC_CACHE:
        _NC_CACHE[key] = build_nc(**kw)
    return _NC_CACHE[key]


KCFG = {"mm": "bf16", "dt_x": BF16, "dt_pt": BF16, "dt_acc": BF16,
        "np_x": ml_dtypes.bfloat16, "np_pt": ml_dtypes.bfloat16}


def kernel(Q, K, V, w_q, w_k, w_v, w_o):
    Q = np.asarray(Q, dtype=np.float32)
    K = np.asarray(K, dtype=np.float32)
    V = np.asarray(V, dtype=np.float32)
    w_q = np.asarray(w_q, dtype=np.float32)
    w_k = np.asarray(w_k, dtype=np.float32)
    w_v = np.asarray(w_v, dtype=np.float32)
    w_o = np.asarray(w_o, dtype=np.float32)

    nc = _get_nc((KCFG["mm"], str(KCFG["dt_x"])),
                 n=N_TOKENS, mm=KCFG["mm"], dt_x=KCFG["dt_x"],
                 dt_pt=KCFG["dt_pt"], dt_acc=KCFG["dt_acc"])
    in_maps = [
        prep_core_inputs(Q, K, V, w_q, w_k, w_v, w_o, c,
                         np_x=KCFG["np_x"], np_pt=KCFG["np_pt"])
        for c in range(N_CORES)
    ]
    res = bass_utils.run_bass_kernel_spmd(nc, in_maps,
                                          core_ids=list(range(N_CORES)))
    out = np.zeros((B, N_TOKENS, D_MODEL), dtype=np.float32)
    for c in range(N_CORES):
        out[c // 4] += res.results[c]["y"].T
    return out


# revision 86
# speedup vs baseline: 1.1944x; 1.0209x over previous
"""Trainium2 Bass kernel for causal MHA (b=2, n=4096, d_model=768, 12 heads).

Sharding: 8 cores = 2 batches x 4 head-groups (3 heads each).
Each core:
  - receives its batch's Q/K/V pre-transposed ([768, n], d_model on rows)
    plus its head-group's weight slices (also pre-transposed on host).
  - projects qT/kT ([64, n] per head, head dim on partitions) and
    v ([n, 64] per head, tokens on partitions) on-chip.
  - computes scoresT[k, q] = kT^T @ qT for PAIRS of 128-key blocks into a
    single [128, 1024] two-bank PSUM tile, exponentiates the pair with ONE
    activation instruction (halves the per-instruction ACT overhead),
    masks causal-boundary blocks with a precomputed 0/1 mask, and
    accumulates outT_aug[65, q] += [v | ones]^T @ P in PSUM.  Row 64 is
    the softmax denominator.
  - normalizes per query-chunk: 1/denom via the fast custom-DVE
    reciprocal, replicated across 64 partitions with a K=1 PE matmul
    (ones[1,64]^T @ r1[1,512] -> PSUM), then one tensor_mul into outT.
    No DRAM round-trips.
  - applies the output projection with its w_o row-slice; host sums the
    4 partial outputs per batch (row-parallel linear unshard).

The phases are software-pipelined: projection chunk t+1, v-projection
blocks, and the output projection for chunk j-1 are issued between
attention chunks so the PE fills the slack of the ACT-bound attention
inner loop and the HAM clock gate stays warm (2.4 GHz).

Weight-column host layout packs the six 64-wide q/k heads into three full
128-row M-blocks ([q0;q1], [q2;k2], [k0;k1]); k2/q2 are then DMA-copied
into a fourth block so every head's scores matmul sees its qT and kT at
the same partition base (a matmul constraint), with h2's operands at BOTH
bases so it can pair with whichever PE row-group is free.
"""

import sys

for _p in ("/opt/trn_rl_repo",):
    if _p not in sys.path:
        sys.path.insert(0, _p)

import numpy as np
import ml_dtypes

import concourse.bass as bass  # noqa: F401  (registers engine classes)
import concourse.tile as tile
from concourse import bacc, mybir
import concourse.bass_utils as bass_utils

P = 128
D_MODEL = 768
KO = D_MODEL // P  # 6 contraction chunks of 128
N_HEADS = 12
D_K = 64
N_CORES = 8
H_LOCAL = 3  # heads per core
D_LOCAL = H_LOCAL * D_K  # 192
B = 2
N_TOKENS = 4096
NQ = 512  # query-chunk size (one PSUM bank of fp32)
NT = 512  # token chunk for q/k projection

F32 = mybir.dt.float32
BF16 = mybir.dt.bfloat16
F32R = mybir.dt.float32r
F8 = mybir.dt.float8e4  # TRN e4m3, max normal 240

# exp(s + ln(1/8)): keeps exp outputs < 240/8 so they fit fp8 e4m3; the
# constant cancels exactly in the softmax division.
EXP_BIAS_FP8 = float(np.log(1.0 / 8.0))

DEBUG = False


def _mm(ap, flavor):
    """View an fp32 AP as the matmul input dtype."""
    if flavor == "f32r":
        return ap.bitcast(F32R)
    return ap


def build_nc(n=N_TOKENS, mm="bf16", dt_x=BF16, dt_pt=BF16, dt_acc=BF16):
    assert n % NQ == 0 and n % NT == 0 and n % P == 0
    # fp8 pt enables DoubleRow PV matmuls (key-block pair fused, K=256)
    dr = dt_pt == F8
    exp_bias = EXP_BIAS_FP8 if dr else 0.0
    nc = bacc.Bacc("TRN2", target_bir_lowering=False, debug=False,
                   num_devices=N_CORES)

    qt_d = nc.dram_tensor("qt", [D_MODEL, n], dt_x, kind="ExternalInput")
    kt_d = nc.dram_tensor("kt", [D_MODEL, n], dt_x, kind="ExternalInput")
    vt_d = nc.dram_tensor("vt", [D_MODEL, n], dt_x, kind="ExternalInput")
    wqk_d = nc.dram_tensor("wqk", [D_MODEL, 2 * D_LOCAL], dt_x,
                           kind="ExternalInput")
    wv_d = nc.dram_tensor("wv", [D_MODEL, D_LOCAL], dt_x, kind="ExternalInput")
    wo_d = nc.dram_tensor("wo", [D_LOCAL, D_MODEL], dt_x, kind="ExternalInput")
    cm_d = nc.dram_tensor("cmask", [P, P], dt_pt, kind="ExternalInput")
    # transposed output [d_model, n]: the host transposes after gather —
    # this lets the output projection keep w_o stationary (12 weight loads
    # total) and stream outT as the moving operand.
    y_d = nc.dram_tensor("y", [D_MODEL, n], F32, kind="ExternalOutput")
    if DEBUG:
        dbg_d = nc.dram_tensor("dbg", [3 * (n // NQ) * H_LOCAL, NQ], F32,
                               kind="ExternalOutput")

    qt_r = qt_d.ap().rearrange("(ko ki) t -> ki ko t", ki=P)
    kt_r = kt_d.ap().rearrange("(ko ki) t -> ki ko t", ki=P)
    vt_r = vt_d.ap().rearrange("(ko ki) t -> ki ko t", ki=P)
    wqk_r = wqk_d.ap().rearrange("(ko ki) m -> ki ko m", ki=P)
    wv_r = wv_d.ap().rearrange("(ko ki) m -> ki ko m", ki=P)

    TCH = n // NT       # q/k projection token chunks
    TB = n // P         # 128-token blocks
    QCH = n // NQ       # query chunks
    KB_PER_Q = NQ // P  # key blocks per query chunk (4)

    # Host weight-column order: [q0 q1 | q2 k2 | k0 k1] -> 3 full M-blocks.
    # qkT_sb blk3 holds DMA-shifted copies: [0:64]=k2, [64:128]=q2.
    q_loc = {0: (0, 0), 1: (64, 0), 2: (0, 1)}
    k_loc = {0: (0, 2), 1: (64, 2), 2: (0, 3)}

    # one raw PSUM bank for HAM keep-alive dummy matmuls (outside the
    # tile pools; only the PE ever touches it, in queue order)
    ka_t = nc.alloc_psum_tensor("keepalive", [64, 96], F32)

    with tile.TileContext(nc) as tc:
        with tc.tile_pool(name="const", bufs=1) as cpool, \
             tc.tile_pool(name="persist", bufs=1) as ppool, \
             tc.tile_pool(name="xqk", bufs=4) as xpool, \
             tc.tile_pool(name="xv", bufs=6) as xvpool, \
             tc.tile_pool(name="pt", bufs=12) as ptpool, \
             tc.tile_pool(name="ysb", bufs=3) as ypool, \
             tc.tile_pool(name="rcp", bufs=4) as rpool, \
             tc.tile_pool(name="ot", bufs=2) as otpool, \
             tc.tile_pool(name="pp_sc", bufs=4, space="PSUM") as pp_sc, \
             tc.tile_pool(name="pp_out", bufs=1, space="PSUM") as pp_out:

            # ---- constants ----
            # DMA issue order = sync-queue order: the first projection
            # needs wqk and the first q/k chunk, so those go first; wv/wo/
            # cm follow behind them.
            wqk_sb = cpool.tile([P, KO, 2 * D_LOCAL], dt_x)
            nc.sync.dma_start(wqk_sb[:], wqk_r)
            wv_sb = cpool.tile([P, KO, D_LOCAL], dt_x)
            wo_sb = cpool.tile([P, 2, D_MODEL], dt_x)
            cm_sb = cpool.tile([P, P], dt_pt)
            ones_sb = cpool.tile([1, P], dt_x)
            nc.vector.memset(ones_sb[:], 1.0)
            ebias_sb = cpool.tile([P, 1], F32)
            nc.vector.memset(ebias_sb[:], exp_bias)

            def late_const_dmas():
                nc.sync.dma_start(wv_sb[:], wv_r)
                # w_o rows: chunk0 = h0,h1 dims (128 rows), chunk1 = h2 (64)
                nc.sync.dma_start(wo_sb[:, 0, :], wo_d.ap()[0:P, :])
                nc.sync.dma_start(wo_sb[0:64, 1, :], wo_d.ap()[P:D_LOCAL, :])
                nc.sync.dma_start(cm_sb[:], cm_d.ap())

            # ---- HAM warm-up ----
            # Dependency-free matmuls run during the initial input-DMA
            # wait, holding the PE busy past the 3.4us activity window so
            # the clock gate opens (1.2 -> 2.4 GHz) before real work lands
            # — and keep it open until the first q/k chunk has landed.
            wus = [pp_sc.tile([P, NQ], F32, tag="psc", name=f"wu{i}")
                   for i in range(2)]
            for i in range(110):
                # alternate PSUM banks so the dummies stream back-to-back
                nc.tensor.matmul(wus[i % 2][0:64, 0:96],
                                 _mm(ones_sb[:, 0:64], mm),
                                 _mm(ones_sb[:, 0:96], mm),
                                 start=True, stop=True,
                                 skip_group_check=True)

            # ---- persistent activations ----
            qkT_sb = ppool.tile([P, 4, n], dt_acc)
            # v slab padded to 80 elements so the key-block stride is a
            # multiple of 16 bytes in fp8 (DoubleRow weight-AP constraint)
            v_sb = ppool.tile([P, TB, H_LOCAL, 80], dt_pt)
            outT_sb = ppool.tile([P, 2, n], dt_acc)
            nc.vector.memset(v_sb[:, :, :, 64:65], 1.0)

            # ---- q/k projection for one 512-token chunk ----
            def proj_dma(t):
                xq = xpool.tile([P, KO, NT], dt_x, tag="x")
                nc.sync.dma_start(xq[:], qt_r[:, :, t * NT:(t + 1) * NT])
                xk = xpool.tile([P, KO, NT], dt_x, tag="x")
                nc.sync.dma_start(xk[:], kt_r[:, :, t * NT:(t + 1) * NT])
                return xq, xk

            def proj_blk(t, xq, xk, blk):
                ps = pp_sc.tile([P, NQ], F32, tag="psc")
                for ko in range(KO):
                    # blk1 contracts q2 against Q-input and k2 against
                    # K-input: split into two half-partition matmuls.
                    if blk == 1:
                        nc.tensor.matmul(
                            ps[0:64, 0:NT],
                            _mm(wqk_sb[:, ko, 128:192], mm),
                            _mm(xq[:, ko, :], mm),
                            start=(ko == 0), stop=(ko == KO - 1),
                            skip_group_check=True,
                        )
                        nc.tensor.matmul(
                            ps[64:128, 0:NT],
                            _mm(wqk_sb[:, ko, 192:256], mm),
                            _mm(xk[:, ko, :], mm),
                            start=(ko == 0), stop=(ko == KO - 1),
                            skip_group_check=True,
                        )
                    else:
                        x = xq if blk == 0 else xk
                        nc.tensor.matmul(
                            ps[:, 0:NT],
                            _mm(wqk_sb[:, ko, blk * 128:(blk + 1) * 128], mm),
                            _mm(x[:, ko, :], mm),
                            start=(ko == 0), stop=(ko == KO - 1),
                        )
                nc.vector.tensor_copy(
                    out=qkT_sb[:, blk, t * NT:(t + 1) * NT],
                    in_=ps[:, 0:NT],
                )
                if blk == 1:
                    # Partition-shifted copies so h2's scores matmul sees
                    # qT/kT at the same base — and at BOTH bases, so h2 can
                    # alternate row-groups and pair with whichever half is
                    # free:
                    #   blk3[0:64]   = k2 (from blk1[64:128])
                    #   blk3[64:128] = q2 (from blk1[0:64])
                    nc.gpsimd.dma_start(
                        qkT_sb[0:64, 3, t * NT:(t + 1) * NT],
                        qkT_sb[64:128, 1, t * NT:(t + 1) * NT],
                    )
                    nc.gpsimd.dma_start(
                        qkT_sb[64:128, 3, t * NT:(t + 1) * NT],
                        qkT_sb[0:64, 1, t * NT:(t + 1) * NT],
                    )

            # ---- v projection (token-major layout) for one 128-token block --
            def v_dma(tb):
                xv = xvpool.tile([P, KO, P], dt_x)
                nc.sync.dma_start(xv[:], vt_r[:, :, tb * P:(tb + 1) * P])
                return xv

            def v_blk(tb, xv):
                ps = pp_sc.tile([P, NQ], F32, tag="psc")
                for ko in range(KO):
                    nc.tensor.matmul(
                        ps[:, 0:D_LOCAL],
                        _mm(xv[:, ko, :], mm),
                        _mm(wv_sb[:, ko, :], mm),
                        start=(ko == 0), stop=(ko == KO - 1),
                    )
                # one copy for all 3 heads: free pattern [3, 64]
                nc.vector.tensor_copy(
                    out=v_sb[:, tb, :, 0:64],
                    in_=ps[:, 0:D_LOCAL].rearrange("p (h d) -> p h d",
                                                   h=H_LOCAL),
                )

            # ---- causal attention, transposed-score flash style ----
            # Heads are interleaved so the PE runs two concurrent score
            # matmuls on disjoint row-groups: h0 lives at partitions 0-63,
            # h1 at 64-127, h2 alternates base per key-block (its qT/kT are
            # replicated at both bases in blk1/blk3).
            def h2_qk(kb):
                if kb % 2 == 0:
                    return (0, 1), (0, 3)   # q2 @ blk1[0:64], k2' @ blk3[0:64]
                return (64, 3), (64, 1)     # q2' @ blk3[64:128], k2 @ blk1[64:128]

            def qk_for(h, kb):
                if h == 2:
                    return h2_qk(kb)
                return q_loc[h], k_loc[h]

            def attn_chunk(j, filler, lowq, pre_pv=None):
                """filler/lowq: lists of thunks issuing ~1-3us of
                non-attention PE work each; one is drained after each
                key-block pair so the PE fills the slack of the ACT-bound
                inner loop instead of stalling ACT with a solid block
                between chunks.  filler holds work the NEXT chunk reads
                (force-drained by the caller); lowq holds deferrable
                output-projection work.  pre_pv (the previous chunk's
                normalization) is issued after pair 0's exps so its DVE
                chain hides under ACT work — it must precede pair 0's PV
                matmuls, which recycle the po tiles it reads."""
                po = [pp_out.tile([P, NQ], F32, tag=f"po{h}", name=f"po{h}")
                      for h in range(H_LOCAL)]
                nkb = KB_PER_Q * j + KB_PER_Q

                def keep_alive():
                    # tiny dependency-free matmul: fills PE wait slots so
                    # the HAM activity window never sees the PE idle and
                    # the 2.4 GHz clock gate stays open.
                    nc.tensor.matmul(ka_t.ap()[0:64, 0:96],
                                     _mm(ones_sb[:, 0:64], mm),
                                     _mm(ones_sb[:, 0:96], mm),
                                     start=True, stop=True,
                                     skip_group_check=True)

                def pv_mms(pta, ptb, ka, kb_, offa, offb):
                    # interleave heads so consecutive matmuls write
                    # different PSUM banks (no same-bank write pressure)
                    for h in range(H_LOCAL):
                        nc.tensor.matmul(
                            po[h][0:65, offa:],
                            _mm(v_sb[:, ka, h, 0:65], mm),
                            _mm(pta[h][:, offa:], mm),
                            start=(ka == 0), stop=False,
                        )
                    for h in range(H_LOCAL):
                        nc.tensor.matmul(
                            po[h][0:65, offb:],
                            _mm(v_sb[:, kb_, h, 0:65], mm),
                            _mm(ptb[h][:, offb:], mm),
                            start=False, stop=(kb_ == nkb - 1),
                        )

                prev = None
                for kb2 in range(0, nkb, 2):
                    ka, kb_ = kb2, kb2 + 1
                    offa = max(ka - KB_PER_Q * j, 0) * P
                    offb = max(kb_ - KB_PER_Q * j, 0) * P
                    diag_a = ka >= KB_PER_Q * j
                    diag_b = kb_ >= KB_PER_Q * j

                    def score_mm(h, kb, psc, off):
                        (qp, qb), (kp, kbblk) = qk_for(h, kb)
                        nc.tensor.matmul(
                            psc[:, off:],
                            _mm(qkT_sb[kp:kp + 64, kbblk,
                                       kb * P:(kb + 1) * P], mm),
                            _mm(qkT_sb[qp:qp + 64, qb,
                                       j * NQ + off:(j + 1) * NQ], mm),
                            start=True, stop=True, skip_group_check=True,
                        )

                    def exp_and_mask(psc, off, diag):
                        pt = ptpool.tile([P, NQ], dt_pt, name="pt")
                        nc.scalar.activation(
                            pt[:, off:], psc[:, off:],
                            mybir.ActivationFunctionType.Exp,
                            bias=ebias_sb[:])
                        if diag:
                            nc.vector.tensor_mul(
                                out=pt[:, off:off + P],
                                in0=pt[:, off:off + P], in1=cm_sb[:])
                        return pt

                    # Single-bank score tiles in a 4-deep ring: h0/h1 fill
                    # all 4 slots, their exps free slots for h2, and every
                    # ring-reuse dependency lands on an exp that was issued
                    # earlier in program order — the PE never waits long.
                    p0a = pp_sc.tile([P, NQ], F32, tag="psc")
                    p1a = pp_sc.tile([P, NQ], F32, tag="psc")
                    score_mm(0, ka, p0a, offa)
                    score_mm(1, ka, p1a, offa)
                    p0b = pp_sc.tile([P, NQ], F32, tag="psc")
                    p1b = pp_sc.tile([P, NQ], F32, tag="psc")
                    score_mm(0, kb_, p0b, offb)
                    score_mm(1, kb_, p1b, offb)
                    pta = [None, None, None]
                    ptb = [None, None, None]
                    pta[0] = exp_and_mask(p0a, offa, diag_a)
                    pta[1] = exp_and_mask(p1a, offa, diag_a)
                    p2a = pp_sc.tile([P, NQ], F32, tag="psc")
                    score_mm(2, ka, p2a, offa)
                    p2b = pp_sc.tile([P, NQ], F32, tag="psc")
                    score_mm(2, kb_, p2b, offb)
                    ptb[0] = exp_and_mask(p0b, offb, diag_b)
                    ptb[1] = exp_and_mask(p1b, offb, diag_b)
                    pta[2] = exp_and_mask(p2a, offa, diag_a)
                    ptb[2] = exp_and_mask(p2b, offb, diag_b)
                    keep_alive()
                    if kb2 == 0 and pre_pv is not None:
                        pre_pv()
                    # two-stage pipeline: the previous pair's PV matmuls
                    # are issued only now, so they never head-of-line
                    # block the PE waiting on their exps.
                    if prev is not None:
                        pv_mms(*prev)
                        if filler:
                            filler.pop(0)()
                        elif lowq:
                            lowq.pop(0)()
                    prev = (pta, ptb, ka, kb_, offa, offb)
                pv_mms(*prev)
                if filler:
                    filler.pop(0)()
                elif lowq:
                    lowq.pop(0)()
                # DVE half of this chunk's normalization: reciprocals are
                # ready well before the next chunk's pre_pv needs them.
                r1s = []
                for h in range(H_LOCAL):
                    # the custom-DVE reciprocal misreads PSUM operands on
                    # real HW (sim allows it) — stage the denominator row
                    # in SBUF first.
                    dn = rpool.tile([1, NQ], F32, tag="dn", name="dn")
                    nc.vector.tensor_copy(out=dn[:], in_=po[h][64:65, :])
                    r1 = rpool.tile([1, NQ], F32, tag="r1", name="r1")
                    nc.vector.reciprocal_approx_fast(out=r1[:], in_=dn[:])
                    # bf16 copy so the broadcast matmul runs on the fast
                    # matmul path (fp32 moving operands are slower)
                    r1b = rpool.tile([1, NQ], dt_x, tag="r1b", name="r1b")
                    nc.vector.tensor_copy(out=r1b[:], in_=r1[:])
                    r1s.append(r1b)
                return po, r1s

            # ---- finish normalizing chunk j: broadcast 1/denom, multiply --
            def norm_chunk(j, po, r1s):
                for h in range(H_LOCAL):
                    # replicate 1/denom across 64 partitions on the (idle)
                    # GPSIMD engine; the multiply reads po from PSUM.
                    rr = rpool.tile([64, NQ], dt_x, tag="rrsb", name="rrsb")
                    nc.gpsimd.partition_broadcast(rr[:], r1s[h][:])
                    if h == 1:
                        # h1 lives at partitions 64-127 of outT blk0; DVE
                        # lanes are partition-locked, so write a temp at
                        # base 0 and DMA partition-shift it up.
                        ot = otpool.tile([64, NQ], dt_acc, name="ot")
                        nc.vector.tensor_mul(out=ot[:], in0=po[h][0:64, :],
                                             in1=rr[:])
                        nc.gpsimd.dma_start(
                            outT_sb[64:128, 0, j * NQ:(j + 1) * NQ], ot[:])
                    else:
                        dst = outT_sb[0:64, 0 if h == 0 else 1,
                                      j * NQ:(j + 1) * NQ]
                        nc.vector.tensor_mul(out=dst, in0=po[h][0:64, :],
                                             in1=rr[:])

            # ---- output projection: two 128-outdim blocks per call, for
            # the 512-token slice of chunk j.  yT[od, t] = wo.T @ outT.
            def outproj_ob(j, ob2):
                for ob in (2 * ob2, 2 * ob2 + 1):
                    ps = pp_sc.tile([P, NQ], F32, tag="psc")
                    nc.tensor.matmul(
                        ps[:],
                        _mm(wo_sb[:, 0, ob * P:(ob + 1) * P], mm),
                        _mm(outT_sb[:, 0, j * NQ:(j + 1) * NQ], mm),
                        start=True, stop=False, skip_group_check=True,
                    )
                    nc.tensor.matmul(
                        ps[:],
                        _mm(wo_sb[0:64, 1, ob * P:(ob + 1) * P], mm),
                        _mm(outT_sb[0:64, 1, j * NQ:(j + 1) * NQ], mm),
                        start=False, stop=True, skip_group_check=True,
                    )
                    ysb = ypool.tile([P, NQ], F32)
                    nc.vector.tensor_copy(out=ysb[:], in_=ps[:])
                    nc.sync.dma_start(
                        y_d.ap()[ob * P:(ob + 1) * P, j * NQ:(j + 1) * NQ],
                        ysb[:])

            # ---- software-pipelined schedule ----
            # prologue: first q/k chunk (blk1 last — h0/h1 scores only need
            # blk0+blk2) and the first 4 v blocks.
            xq0, xk0 = proj_dma(0)
            late_const_dmas()
            xvs = [v_dma(tb) for tb in range(min(KB_PER_Q, TB))]
            for blk in (0, 2, 1):
                proj_blk(0, xq0, xk0, blk)
            # v blocks 0-1 feed chunk 0's first pair and must precede it;
            # 2-3 (used by its second pair) slot in as its first filler.
            for tb, xv in list(enumerate(xvs))[:2]:
                v_blk(tb, xv)
            lowq = []
            po_prev = None
            for j in range(QCH):
                # stage next round's inputs and build its filler list
                filler = []
                if j == 0:
                    def _v23(rest=list(enumerate(xvs))[2:]):
                        for tb, xv in rest:
                            v_blk(tb, xv)
                    filler.append(_v23)
                if j + 1 < TCH:
                    xq, xk = proj_dma(j + 1)
                    for blk in (0, 2, 1):
                        filler.append(
                            lambda t=j + 1, a=xq, b=xk, bl=blk:
                            proj_blk(t, a, b, bl))
                for tb in range(KB_PER_Q * (j + 1),
                                min(KB_PER_Q * (j + 2), TB)):
                    xv = v_dma(tb)
                    filler.append(lambda t=tb, x=xv: v_blk(t, x))
                if j > 0:
                    for ob2 in range(3):
                        lowq.append(lambda jj=j - 1, o=ob2:
                                    outproj_ob(jj, o))
                pre_pv = None
                if po_prev is not None:
                    pre_pv = (lambda jj=j - 1, pp=po_prev:
                              norm_chunk(jj, *pp))
                po_prev = attn_chunk(j, filler, lowq, pre_pv)
                # next chunk reads these — they must be issued before it
                for fn in filler:
                    fn()
            norm_chunk(QCH - 1, *po_prev)
            for fn in lowq:
                fn()
            for ob2 in range(3):
                outproj_ob(QCH - 1, ob2)

    nc.compile()
    return nc


def make_causal_mask_np(dt=np.float32):
    """[128, 128] lower-left keep mask: m[p, f] = 1.0 iff f >= p."""
    f = np.arange(P)[None, :]
    p = np.arange(P)[:, None]
    return (f >= p).astype(np.float32).astype(dt)


def prep_core_inputs(Q, K, V, w_q, w_k, w_v, w_o, core, n=N_TOKENS,
                     np_x=ml_dtypes.bfloat16, np_pt=ml_dtypes.bfloat16):
    """Host-side sharding/layout prep for one core. All fp32 numpy in."""
    b = core // 4
    g = core % 4
    hs = g * D_LOCAL
    scale = 1.0 / np.sqrt(D_K)
    qt = np.ascontiguousarray(Q[b].T).astype(np_x)
    kt = np.ascontiguousarray(K[b].T).astype(np_x)
    vt = np.ascontiguousarray(V[b].T).astype(np_x)
    wql = w_q[hs:hs + D_LOCAL] * scale
    wkl = w_k[hs:hs + D_LOCAL]
    # column order [q0 q1 | q2 k2 | k0 k1] (see build_nc)
    wqk = np.ascontiguousarray(
        np.concatenate([wql[0:128], wql[128:192], wkl[128:192], wkl[0:128]],
                       axis=0).T
    ).astype(np_x)
    wv = np.ascontiguousarray(w_v[hs:hs + D_LOCAL].T).astype(np_x)
    wo = np.ascontiguousarray(w_o[:, hs:hs + D_LOCAL].T).astype(np_x)
    cm = make_causal_mask_np(np_pt)
    return {"qt": qt, "kt": kt, "vt": vt, "wqk": wqk, "wv": wv, "wo": wo,
            "cmask": cm}


_NC_CACHE = {}


def _get_nc(key, **kw):
    if key not in _NC_CACHE:
        _NC_CACHE[key] = build_nc(**kw)
    return _NC_CACHE[key]


KCFG = {"mm": "bf16", "dt_x": BF16, "dt_pt": BF16, "dt_acc": BF16,
        "np_x": ml_dtypes.bfloat16, "np_pt": ml_dtypes.bfloat16}


def kernel(Q, K, V, w_q, w_k, w_v, w_o):
    Q = np.asarray(Q, dtype=np.float32)
    K = np.asarray(K, dtype=np.float32)
    V = np.asarray(V, dtype=np.float32)
    w_q = np.asarray(w_q, dtype=np.float32)
    w_k = np.asarray(w_k, dtype=np.float32)
    w_v = np.asarray(w_v, dtype=np.float32)
    w_o = np.asarray(w_o, dtype=np.float32)

    nc = _get_nc((KCFG["mm"], str(KCFG["dt_x"])),
                 n=N_TOKENS, mm=KCFG["mm"], dt_x=KCFG["dt_x"],
                 dt_pt=KCFG["dt_pt"], dt_acc=KCFG["dt_acc"])
    in_maps = [
        prep_core_inputs(Q, K, V, w_q, w_k, w_v, w_o, c,
                         np_x=KCFG["np_x"], np_pt=KCFG["np_pt"])
        for c in range(N_CORES)
    ]
    res = bass_utils.run_bass_kernel_spmd(nc, in_maps,
                                          core_ids=list(range(N_CORES)))
    out = np.zeros((B, N_TOKENS, D_MODEL), dtype=np.float32)
    for c in range(N_CORES):
        out[c // 4] += res.results[c]["y"].T
    return out
